# revision 12
# baseline (speedup 1.0000x reference)
"""MultiHeadSectionAttentionImputer on 8 TRN2 NeuronCores (Bass/Tile).

Sharding: the N=6144 existing sections are split across the 8 cores
(768 each). Each core:
  - projects its exist-shard to K,V  (K_loc = X_e @ Wk, V_loc = X_e @ Wv + ones col)
  - projects the full missing set to Q (duplicated across cores; Wq,bq
    pre-scaled by 1/sqrt(d_k) on host)
  - computes scoresT[n,m] per head with a fused 128-deep contraction:
      d' = [q-dims(64) | cooc-bias-dims(64)]  ->  q.k/sqrt(dk) + mb.eb
  - exp() without max subtraction (scores are bounded ~<60; fp32 range ok)
  - partial out^T = attn @ [V | 1]  ->  numerator (64 cols) + denominator
Host combines partial numerators/denominators across cores (softmax over
the full key set), adds bv, and scatters into a copy of ehr_embeddings.

All matmul inputs are float32r (tf32-like, full-rate on PE) except the
attention-weight matmul which uses bf16 (exp output cast, attn in [0, e^60]).
"""

import os
import sys
import numpy as np
from contextlib import ExitStack

sys.path.insert(0, "/opt/trn_rl_repo")

# problem constants (hardcoded; kernel.py must be self-contained)
H = 12          # heads
DK = 64         # head dim
E = 768         # embed dim
TOTAL = H * DK  # 768
M = 2048        # missing sections
N = 6144        # existing sections
S = 8192        # total sections
CORES = 8
NLOC = N // CORES        # 768 keys per core
EC = E // 128            # 6 contraction chunks
NI = NLOC // 128         # 6 key chunks per core
MI = M // 128            # 16 query chunks
PAIRS = H // 2           # 6 head pairs

_CACHE = {}
LAST_EXEC_NS = None
LAST_TRACE_DIR = None


def _build():
    import concourse.bass as bass
    import concourse.tile as tile
    from concourse import bacc, mybir

    F32 = mybir.dt.float32
    FP16 = mybir.dt.float16
    BF16 = mybir.dt.bfloat16
    Exp = mybir.ActivationFunctionType.Exp

    nc = bacc.Bacc("TRN2", target_bir_lowering=False, debug=False)

    # ---- I/O ----
    xt_m = nc.dram_tensor("xt_m", [E, M], FP16, kind="ExternalInput").ap()
    mbt = nc.dram_tensor("mbt", [H * DK, M], FP16, kind="ExternalInput").ap()
    xt_e = nc.dram_tensor("xt_e", [E, NLOC], FP16, kind="ExternalInput").ap()
    ebt = nc.dram_tensor("ebt", [H * DK, NLOC], FP16, kind="ExternalInput").ap()
    wq = nc.dram_tensor("wq", [E, TOTAL], FP16, kind="ExternalInput").ap()
    wk = nc.dram_tensor("wk", [E, TOTAL], FP16, kind="ExternalInput").ap()
    wv = nc.dram_tensor("wv", [E, TOTAL], FP16, kind="ExternalInput").ap()
    bq = nc.dram_tensor("bq", [128, PAIRS], F32, kind="ExternalInput").ap()
    out_p = nc.dram_tensor("out_p", [H, M, DK + 1], F32, kind="ExternalOutput").ap()

    with tile.TileContext(nc) as tc, ExitStack() as ctx:
        persist = ctx.enter_context(tc.tile_pool(name="persist", bufs=1))
        qpt_pool = ctx.enter_context(tc.tile_pool(name="qpt", bufs=4))
        attn_pool = ctx.enter_context(tc.tile_pool(name="attn", bufs=15))
        osb_pool = ctx.enter_context(tc.tile_pool(name="osb", bufs=4))
        proj_ps = ctx.enter_context(tc.tile_pool(name="proj_ps", bufs=1, space="PSUM"))
        sc_ps = ctx.enter_context(tc.tile_pool(name="sc_ps", bufs=2, space="PSUM"))
        av_ps = ctx.enter_context(tc.tile_pool(name="av_ps", bufs=3, space="PSUM"))

        # ---- stage inputs in SBUF ----
        xtm_sb = []   # 6 x [128, M]
        xte_sb = []   # 6 x [128, NLOC]
        wq_sb = []    # 6 x [128, TOTAL]
        wk_sb = []
        wv_sb = []
        # input DMAs spread across queues, pieces ordered first-needed-first.
        # Critical path budget (descriptor cadence ~0.7-1.5us/queue):
        #   sync:   wk (6)                      -> kt0 lhsT
        #   scalar: bq, xte piece0, xtm piece0  -> kt0/qt0 rhs
        #   gpsimd: mbt h0/h1, ebt p0, wq, wv   -> q0 rows, qt lhsT, v
        # then the remaining pieces trickle on sync/scalar.
        bq_sb = persist.tile([128, PAIRS], F32, tag="bq")
        nc.scalar.dma_start(bq_sb[:], bq)
        for ec in range(EC):
            t = persist.tile([128, TOTAL], FP16, tag=f"wk{ec}", name=f"wk{ec}")
            nc.sync.dma_start(t[:], wk[ec * 128:(ec + 1) * 128, :])
            wk_sb.append(t)
            t = persist.tile([128, NLOC], FP16, tag=f"xte{ec}", name=f"xte{ec}")
            nc.scalar.dma_start(t[:, 0:512], xt_e[ec * 128:(ec + 1) * 128, 0:512])
            xte_sb.append(t)
            t = persist.tile([128, M], FP16, tag=f"xtm{ec}", name=f"xtm{ec}")
            nc.scalar.dma_start(t[:, 0:512], xt_m[ec * 128:(ec + 1) * 128, 0:512])
            xtm_sb.append(t)
            t = persist.tile([128, TOTAL], FP16, tag=f"wq{ec}", name=f"wq{ec}")
            nc.gpsimd.dma_start(t[:], wq[ec * 128:(ec + 1) * 128, :])
            wq_sb.append(t)
            t = persist.tile([128, TOTAL], FP16, tag=f"wv{ec}", name=f"wv{ec}")
            nc.gpsimd.dma_start(t[:], wv[ec * 128:(ec + 1) * 128, :])
            wv_sb.append(t)
        # remaining pieces, in qt-quarter / kt-half consumption order
        for mq in range(1, 4):
            for ec in range(EC):
                nc.scalar.dma_start(
                    xtm_sb[ec][:, mq * 512:(mq + 1) * 512],
                    xt_m[ec * 128:(ec + 1) * 128, mq * 512:(mq + 1) * 512])
        for ec in range(EC):
            nc.sync.dma_start(
                xte_sb[ec][:, 512:NLOC], xt_e[ec * 128:(ec + 1) * 128, 512:NLOC])

        # K'T tiles per head [128, NLOC]: rows = k-dims | eb-dims (parity layout)
        kpt = [persist.tile([128, NLOC], FP16, tag=f"kpt{h}", name=f"kpt{h}") for h in range(H)]
        # V tiles per key chunk [128, H, DK+1] bf16 (ones col at [., ., DK])
        vsb = [persist.tile([128, H, DK + 1], BF16, tag=f"v{ni}", name=f"v{ni}") for ni in range(NI)]

        def emit_scores_exp_half(h, qt, ni, half, at):
            """scoresT half-chunk [128 keys, 1024 queries] + exp into attnT.
            Two halves -> the 2-bank scores psum double-buffers, keeping
            ACT busy back-to-back instead of waiting a full 4-matmul round."""
            ps = sc_ps.tile([128, 1024], F32, tag="sc", name="sc_ps_t")
            mo = half * 1024
            for mj in range(2):
                nc.tensor.matmul(
                    ps[:, mj * 512:(mj + 1) * 512],
                    lhsT=kpt[h][:, ni * 128:(ni + 1) * 128],
                    rhs=qt[:, mo + mj * 512:mo + (mj + 1) * 512],
                    start=True, stop=True)
            nc.scalar.activation(at[:, mo:mo + 1024], ps[:], Exp)

        def emit_av(h, attns, mj):
            """out chunks [128 queries, DK+1] for head h, mi in [4mj, 4mj+4).
            attnT chunks are the stationary operand (bf16 -> fast weight
            load); each matmul streams the 65-wide [V | 1] tile."""
            for mi in range(4 * mj, 4 * mj + 4):
                ps = av_ps.tile([128, DK + 1], F32, tag="av", name="av_ps_t")
                for ni in range(NI):
                    nc.tensor.matmul(
                        ps[:], lhsT=attns[ni][:, mi * 128:(mi + 1) * 128],
                        rhs=vsb[ni][:, h, :],
                        start=(ni == 0), stop=(ni == NI - 1))
                ot = osb_pool.tile([128, DK + 1], F32, tag="osb", name="osb_t")
                nc.vector.tensor_copy(ot[:], ps[:])
                nc.sync.dma_start(out_p[h, mi * 128:(mi + 1) * 128, :], ot[:])

        # ---- emission schedule ----
        # Unit-queue: small PE work units (~1.4us each) are drained between
        # scores/exp emissions so the PE fills the exp-wait gaps (scores
        # psum has bufs=1, so s(h,ni+1) waits on exp(h,ni)).
        from collections import deque
        units = deque()
        qts = {}
        pair_ready = {0: 0}  # pair -> emitted kt+qt half count (4 = ready)

        def qt_unit(p, mh):
            def f():
                pair_ready[p] = pair_ready.get(p, 0) + 1
                q0, q1 = qts.get(2 * p), qts.get(2 * p + 1)
                if q0 is None:
                    q0 = qpt_pool.tile([128, M], FP16, tag="qpt", name=f"qpt{2*p}")
                    q1 = qpt_pool.tile([128, M], FP16, tag="qpt", name=f"qpt{2*p+1}")
                    h0, h1 = 2 * p, 2 * p + 1
                    eng = nc.gpsimd if p == 0 else nc.sync
                    eng.dma_start(q0[64:128, :], mbt[h0 * DK:(h0 + 1) * DK, :])
                    eng.dma_start(q1[0:64, :], mbt[h1 * DK:(h1 + 1) * DK, :])
                    qts[2 * p], qts[2 * p + 1] = q0, q1
                emit_qt_half(p, mh, q0, q1)
            return f

        def emit_qt_half(p, mh, q0, q1):
            ps = proj_ps.tile([128, 512], F32, tag="proj", name="proj_qt")
            mo = mh * 512
            for ec in range(EC):
                nc.tensor.matmul(ps[:], lhsT=wq_sb[ec][:, p * 128:(p + 1) * 128],
                                 rhs=xtm_sb[ec][:, mo:mo + 512],
                                 start=(ec == 0), stop=(ec == EC - 1))
            nc.vector.tensor_scalar_add(
                q0[0:64, mo:mo + 512], ps[0:64, :], bq_sb[0:64, p:p + 1])
            nc.vector.tensor_scalar_add(
                q1[64:128, mo:mo + 512], ps[64:128, :], bq_sb[64:128, p:p + 1])

        def kt_unit(p, half):
            def f():
                pair_ready[p] = pair_ready.get(p, 0) + 1
                emit_kt_half(p, half)
            return f

        def emit_kt_half(p, half):
            h0, h1 = 2 * p, 2 * p + 1
            lo, hi = (0, 512) if half == 0 else (512, NLOC)
            ps = proj_ps.tile([128, 512], F32, tag="proj", name="proj_kt")
            for ec in range(EC):
                nc.tensor.matmul(ps[:, 0:hi - lo], lhsT=wk_sb[ec][:, p * 128:(p + 1) * 128],
                                 rhs=xte_sb[ec][:, lo:hi], start=(ec == 0), stop=(ec == EC - 1))
            nc.vector.tensor_copy(kpt[h0][0:64, lo:hi], ps[0:64, 0:hi - lo])
            nc.vector.tensor_copy(kpt[h1][64:128, lo:hi], ps[64:128, 0:hi - lo])
            if half == 0:
                eng = nc.gpsimd if p == 0 else nc.sync
                eng.dma_start(kpt[h0][64:128, :], ebt[h0 * DK:(h0 + 1) * DK, :])
                eng.dma_start(kpt[h1][0:64, :], ebt[h1 * DK:(h1 + 1) * DK, :])

        def v_unit(ni, half):
            def f():
                lo, hi = (0, 512) if half == 0 else (512, TOTAL)
                ps = proj_ps.tile([128, 512], F32, tag="proj", name="proj_v")
                for ec in range(EC):
                    nc.tensor.matmul(ps[:, 0:hi - lo],
                                     lhsT=xte_sb[ec][:, ni * 128:(ni + 1) * 128],
                                     rhs=wv_sb[ec][:, lo:hi], start=(ec == 0), stop=(ec == EC - 1))
                hlo, hhi = lo // DK, hi // DK
                nc.vector.tensor_copy(
                    vsb[ni][:, hlo:hhi, 0:DK],
                    ps[:, 0:hi - lo].rearrange("p (h d) -> p h d", d=DK))
                if half == 1:
                    nc.vector.memset(vsb[ni][:, :, DK], 1.0)
            return f

        def av_unit(h, attns, mj):
            def f():
                emit_av(h, attns, mj)
            return f

        # kt pair0 + qt pair0 emitted up front (head 0 critical path)
        emit_kt_half(0, 0)
        emit_kt_half(0, 1)
        pair_ready[0] = 2
        for mh in range(4):
            qt_unit(0, mh)()
        # v units right after (needed by first av drains in head 1)
        for ni in range(NI):
            units.append(v_unit(ni, 0))
            units.append(v_unit(ni, 1))

        slot = 0
        for h in range(H):
            p = h // 2
            if h % 2 == 1 and p + 1 <= PAIRS - 1:
                units.append(kt_unit(p + 1, 0))
                units.append(kt_unit(p + 1, 1))
                for mh in range(4):
                    units.append(qt_unit(p + 1, mh))
            # this pair's K'T and Q'T must be fully emitted before scores
            # read them (a read emitted before its writer would be silently
            # unordered by the tile tracer)
            while pair_ready.get(p, 0) < 6:
                units.popleft()()
            attns = []
            for ni in range(NI):
                at = attn_pool.tile([128, M], BF16, tag="attn", name="attn_t")
                attns.append(at)
                for half in range(2):
                    emit_scores_exp_half(h, qts[h], ni, half, at)
                    # drain units between exp emissions to fill PE gaps
                    npump = 1 if half == 0 else (
                        2 if (slot < 8 or len(units) > 10) else 1)
                    for _ in range(npump):
                        if units:
                            units.popleft()()
                slot += 1
            qts[h] = None  # allow qpt slot reuse
            for mj in range(4):
                units.append(av_unit(h, attns, mj))
        while units:
            units.popleft()()

    nc.compile()
    return nc


def _get_nc():
    if "nc" not in _CACHE:
        _CACHE["nc"] = _build()
    return _CACHE["nc"]


def kernel(**inputs):
    global LAST_EXEC_NS, LAST_TRACE_DIR
    from concourse.bass_utils import run_bass_kernel_spmd

    ehr = np.asarray(inputs["ehr_embeddings"], dtype=np.float32)
    mi = np.asarray(inputs["missing_indices"]).astype(np.int64)
    ei = np.asarray(inputs["exist_indices"]).astype(np.int64)
    Wq = np.asarray(inputs["Wq"], dtype=np.float32)
    Wk = np.asarray(inputs["Wk"], dtype=np.float32)
    Wv = np.asarray(inputs["Wv"], dtype=np.float32)
    bq = np.asarray(inputs["bq"], dtype=np.float32)
    bv = np.asarray(inputs["bv"], dtype=np.float32)
    cooc = np.asarray(inputs["cooc_bias"], dtype=np.float32)
    # bk is softmax-shift-invariant (adds a per-query constant to scores);
    # dropped on device, consistent across cores so the combine is exact.

    scale = 1.0 / np.sqrt(np.float32(DK))
    wq_s = np.ascontiguousarray((Wq * scale).astype(np.float16))
    bq_s = np.ascontiguousarray((bq * scale).reshape(PAIRS, 128).T)

    missing_emb = ehr[mi]                       # [M, E]
    xt_m = np.ascontiguousarray(missing_emb.T.astype(np.float16))  # [E, M]
    mbt = np.ascontiguousarray(
        cooc[:, mi, :].transpose(0, 2, 1).reshape(H * DK, M).astype(np.float16))

    common = {"xt_m": xt_m, "mbt": mbt, "wq": wq_s,
              "wk": np.ascontiguousarray(Wk.astype(np.float16)),
              "wv": np.ascontiguousarray(Wv.astype(np.float16)), "bq": bq_s}
    in_maps = []
    for c in range(CORES):
        eic = ei[c * NLOC:(c + 1) * NLOC]
        xt_e = np.ascontiguousarray(ehr[eic].T.astype(np.float16))  # [E, NLOC]
        ebt = np.ascontiguousarray(
            cooc[:, eic, :].transpose(0, 2, 1).reshape(H * DK, NLOC).astype(np.float16))
        in_maps.append({**common, "xt_e": xt_e, "ebt": ebt})

    nc = _get_nc()
    trace = os.environ.get("KERNEL_TRACE") == "1"
    kwargs = {}
    if trace:
        import tempfile
        LAST_TRACE_DIR = tempfile.mkdtemp(prefix="kern_trace_")
        kwargs = {"trace": True, "tmpdir": LAST_TRACE_DIR}
        try:
            import ntff_shim
            ntff_shim.install()
        except ImportError:
            pass
    res = run_bass_kernel_spmd(nc, in_maps, list(range(CORES)), **kwargs)
    LAST_EXEC_NS = res.exec_time_ns

    # ---- host combine ----
    num = np.zeros((H, M, DK), dtype=np.float64)
    den = np.zeros((H, M), dtype=np.float64)
    for c in range(CORES):
        op = res.results[c]["out_p"].astype(np.float64)  # [H, M, DK+1]
        num += op[:, :, :DK]
        den += op[:, :, DK]
    out = num / den[:, :, None]                          # [H, M, DK]
    out = out.transpose(1, 0, 2).reshape(M, TOTAL) + bv.astype(np.float64)
    result = ehr.copy()
    result[mi] = out.astype(np.float32)
    return result


# revision 14
# speedup vs baseline: 1.0622x; 1.0622x over previous
"""MultiHeadSectionAttentionImputer on 8 TRN2 NeuronCores (Bass/Tile).

Sharding: the N=6144 existing sections are split across the 8 cores
(768 each). Each core:
  - projects its exist-shard to K,V  (K_loc = X_e @ Wk, V_loc = X_e @ Wv + ones col)
  - projects the full missing set to Q (duplicated across cores; Wq,bq
    pre-scaled by 1/sqrt(d_k) on host)
  - computes scoresT[n,m] per head with a fused 128-deep contraction:
      d' = [q-dims(64) | cooc-bias-dims(64)]  ->  q.k/sqrt(dk) + mb.eb
  - exp() without max subtraction (scores are bounded ~<60; fp32 range ok)
  - partial out^T = attn @ [V | 1]  ->  numerator (64 cols) + denominator
Host combines partial numerators/denominators across cores (softmax over
the full key set), adds bv, and scatters into a copy of ehr_embeddings.

All matmul inputs are float32r (tf32-like, full-rate on PE) except the
attention-weight matmul which uses bf16 (exp output cast, attn in [0, e^60]).
"""

import os
import sys
import numpy as np
from contextlib import ExitStack

sys.path.insert(0, "/opt/trn_rl_repo")

# problem constants (hardcoded; kernel.py must be self-contained)
H = 12          # heads
DK = 64         # head dim
E = 768         # embed dim
TOTAL = H * DK  # 768
M = 2048        # missing sections
N = 6144        # existing sections
S = 8192        # total sections
CORES = 8
NLOC = N // CORES        # 768 keys per core
EC = E // 128            # 6 contraction chunks
NI = NLOC // 128         # 6 key chunks per core
MI = M // 128            # 16 query chunks
PAIRS = H // 2           # 6 head pairs

_CACHE = {}
LAST_EXEC_NS = None
LAST_TRACE_DIR = None


def _build():
    import concourse.bass as bass
    import concourse.tile as tile
    from concourse import bacc, mybir

    F32 = mybir.dt.float32
    FP16 = mybir.dt.float16
    BF16 = mybir.dt.bfloat16
    Exp = mybir.ActivationFunctionType.Exp

    nc = bacc.Bacc("TRN2", target_bir_lowering=False, debug=False)

    # ---- I/O ----
    xt_m = nc.dram_tensor("xt_m", [E, M], FP16, kind="ExternalInput").ap()
    mbt = nc.dram_tensor("mbt", [H * DK, M], FP16, kind="ExternalInput").ap()
    xt_e = nc.dram_tensor("xt_e", [E, NLOC], FP16, kind="ExternalInput").ap()
    ebt = nc.dram_tensor("ebt", [H * DK, NLOC], FP16, kind="ExternalInput").ap()
    wq = nc.dram_tensor("wq", [E, TOTAL], FP16, kind="ExternalInput").ap()
    wk = nc.dram_tensor("wk", [E, TOTAL], FP16, kind="ExternalInput").ap()
    wv = nc.dram_tensor("wv", [E, TOTAL], FP16, kind="ExternalInput").ap()
    bq = nc.dram_tensor("bq", [128, PAIRS], F32, kind="ExternalInput").ap()
    out_p = nc.dram_tensor("out_p", [H, M, DK + 1], F32, kind="ExternalOutput").ap()

    with tile.TileContext(nc) as tc, ExitStack() as ctx:
        persist = ctx.enter_context(tc.tile_pool(name="persist", bufs=1))
        qpt_pool = ctx.enter_context(tc.tile_pool(name="qpt", bufs=4))
        attn_pool = ctx.enter_context(tc.tile_pool(name="attn", bufs=15))
        osb_pool = ctx.enter_context(tc.tile_pool(name="osb", bufs=12))
        proj_ps = ctx.enter_context(tc.tile_pool(name="proj_ps", bufs=1, space="PSUM"))
        sc_ps = ctx.enter_context(tc.tile_pool(name="sc_ps", bufs=2, space="PSUM"))
        av_ps = ctx.enter_context(tc.tile_pool(name="av_ps", bufs=3, space="PSUM"))

        # ---- stage inputs in SBUF ----
        xtm_sb = []   # 6 x [128, M]
        xte_sb = []   # 6 x [128, NLOC]
        wq_sb = []    # 6 x [128, TOTAL]
        wk_sb = []
        wv_sb = []
        # input DMAs spread across queues, pieces ordered first-needed-first.
        # Only pair-0 column slices of wk/wq are on the critical path:
        #   sync:   wk[:,0:128] (kt0 lhsT), wk rest, xte col-rest
        #   scalar: bq, xte piece0 + xtm piece0 (kt0/qt0 rhs), xtm rest
        #   gpsimd: mbt h0/h1 + ebt p0 (q0/k0 bias rows), wq piece0, wv, wq rest
        kpt = [persist.tile([128, NLOC], FP16, tag=f"kpt{h}", name=f"kpt{h}") for h in range(H)]
        vsb = [persist.tile([128, H, DK + 1], BF16, tag=f"v{ni}", name=f"v{ni}") for ni in range(NI)]
        bq_sb = persist.tile([128, PAIRS], F32, tag="bq")
        nc.scalar.dma_start(bq_sb[:], bq)
        q0_0 = qpt_pool.tile([128, M], FP16, tag="qpt", name="qpt0")
        q1_0 = qpt_pool.tile([128, M], FP16, tag="qpt", name="qpt1")
        nc.gpsimd.dma_start(q0_0[64:128, :], mbt[0:DK, :])
        nc.gpsimd.dma_start(q1_0[0:64, :], mbt[DK:2 * DK, :])
        nc.gpsimd.dma_start(kpt[0][64:128, :], ebt[0:DK, :])
        nc.gpsimd.dma_start(kpt[1][0:64, :], ebt[DK:2 * DK, :])
        for ec in range(EC):
            t = persist.tile([128, TOTAL], FP16, tag=f"wk{ec}", name=f"wk{ec}")
            nc.sync.dma_start(t[:, 0:128], wk[ec * 128:(ec + 1) * 128, 0:128])
            wk_sb.append(t)
            t = persist.tile([128, NLOC], FP16, tag=f"xte{ec}", name=f"xte{ec}")
            nc.scalar.dma_start(t[:, 0:512], xt_e[ec * 128:(ec + 1) * 128, 0:512])
            xte_sb.append(t)
            t = persist.tile([128, M], FP16, tag=f"xtm{ec}", name=f"xtm{ec}")
            nc.scalar.dma_start(t[:, 0:512], xt_m[ec * 128:(ec + 1) * 128, 0:512])
            xtm_sb.append(t)
            t = persist.tile([128, TOTAL], FP16, tag=f"wq{ec}", name=f"wq{ec}")
            nc.gpsimd.dma_start(t[:, 0:128], wq[ec * 128:(ec + 1) * 128, 0:128])
            wq_sb.append(t)
        for ec in range(EC):
            t = persist.tile([128, TOTAL], FP16, tag=f"wv{ec}", name=f"wv{ec}")
            nc.gpsimd.dma_start(t[:], wv[ec * 128:(ec + 1) * 128, :])
            wv_sb.append(t)
        # non-critical remainders
        for ec in range(EC):
            nc.sync.dma_start(wk_sb[ec][:, 128:TOTAL],
                              wk[ec * 128:(ec + 1) * 128, 128:TOTAL])
        for ec in range(EC):
            nc.gpsimd.dma_start(wq_sb[ec][:, 128:TOTAL],
                                wq[ec * 128:(ec + 1) * 128, 128:TOTAL])
        for mq in range(1, 4):
            for ec in range(EC):
                nc.scalar.dma_start(
                    xtm_sb[ec][:, mq * 512:(mq + 1) * 512],
                    xt_m[ec * 128:(ec + 1) * 128, mq * 512:(mq + 1) * 512])
        for ec in range(EC):
            nc.sync.dma_start(
                xte_sb[ec][:, 512:NLOC], xt_e[ec * 128:(ec + 1) * 128, 512:NLOC])

        def emit_scores_exp_half(h, qt, ni, half, at):
            """scoresT half-chunk [128 keys, 1024 queries] + exp into attnT.
            Two halves -> the 2-bank scores psum double-buffers, keeping
            ACT busy back-to-back instead of waiting a full 4-matmul round."""
            ps = sc_ps.tile([128, 1024], F32, tag="sc", name="sc_ps_t")
            mo = half * 1024
            for mj in range(2):
                nc.tensor.matmul(
                    ps[:, mj * 512:(mj + 1) * 512],
                    lhsT=kpt[h][:, ni * 128:(ni + 1) * 128],
                    rhs=qt[:, mo + mj * 512:mo + (mj + 1) * 512],
                    start=True, stop=True)
            nc.scalar.activation(at[:, mo:mo + 1024], ps[:], Exp)

        def emit_av(h, attns, g):
            """out chunks [128 queries, DK+1] for head h, mi in [2g, 2g+2)."""
            for mi in range(2 * g, 2 * g + 2):
                ps = av_ps.tile([128, DK + 1], F32, tag="av", name="av_ps_t")
                for ni in range(NI):
                    nc.tensor.matmul(
                        ps[:], lhsT=attns[ni][:, mi * 128:(mi + 1) * 128],
                        rhs=vsb[ni][:, h, :],
                        start=(ni == 0), stop=(ni == NI - 1))
                ot = osb_pool.tile([128, DK + 1], F32, tag="osb", name="osb_t")
                nc.vector.tensor_copy(ot[:], ps[:])
                nc.sync.dma_start(out_p[h, mi * 128:(mi + 1) * 128, :], ot[:])

        # ---- emission schedule ----
        # Unit-queue of (pe_cost_us, fn): drained between scores/exp
        # emissions under a per-sub-slot budget so the PE fills the exp
        # pipeline gaps without pushing the next scores matmuls far back
        # in its (in-order) stream.
        from collections import deque
        units = deque()
        qts = {0: q0_0, 1: q1_0}
        pair_ready = {0: 0}  # pair -> emitted kt+qt piece count (6 = ready)

        def qt_unit(p, mh):
            def f():
                pair_ready[p] = pair_ready.get(p, 0) + 1
                q0, q1 = qts.get(2 * p), qts.get(2 * p + 1)
                if q0 is None:
                    q0 = qpt_pool.tile([128, M], FP16, tag="qpt", name=f"qpt{2*p}")
                    q1 = qpt_pool.tile([128, M], FP16, tag="qpt", name=f"qpt{2*p+1}")
                    h0, h1 = 2 * p, 2 * p + 1
                    nc.sync.dma_start(q0[64:128, :], mbt[h0 * DK:(h0 + 1) * DK, :])
                    nc.sync.dma_start(q1[0:64, :], mbt[h1 * DK:(h1 + 1) * DK, :])
                    qts[2 * p], qts[2 * p + 1] = q0, q1
                emit_qt_half(p, mh, q0, q1)
            return (1.3, f)

        def emit_qt_half(p, mh, q0, q1):
            ps = proj_ps.tile([128, 512], F32, tag="proj", name="proj_qt")
            mo = mh * 512
            for ec in range(EC):
                nc.tensor.matmul(ps[:], lhsT=wq_sb[ec][:, p * 128:(p + 1) * 128],
                                 rhs=xtm_sb[ec][:, mo:mo + 512],
                                 start=(ec == 0), stop=(ec == EC - 1))
            nc.vector.tensor_scalar_add(
                q0[0:64, mo:mo + 512], ps[0:64, :], bq_sb[0:64, p:p + 1])
            nc.vector.tensor_scalar_add(
                q1[64:128, mo:mo + 512], ps[64:128, :], bq_sb[64:128, p:p + 1])

        def kt_unit(p, half):
            def f():
                pair_ready[p] = pair_ready.get(p, 0) + 1
                emit_kt_half(p, half)
            return (1.4, f)

        def emit_kt_half(p, half):
            h0, h1 = 2 * p, 2 * p + 1
            lo, hi = (0, 512) if half == 0 else (512, NLOC)
            ps = proj_ps.tile([128, 512], F32, tag="proj", name="proj_kt")
            for ec in range(EC):
                nc.tensor.matmul(ps[:, 0:hi - lo], lhsT=wk_sb[ec][:, p * 128:(p + 1) * 128],
                                 rhs=xte_sb[ec][:, lo:hi], start=(ec == 0), stop=(ec == EC - 1))
            nc.vector.tensor_copy(kpt[h0][0:64, lo:hi], ps[0:64, 0:hi - lo])
            nc.vector.tensor_copy(kpt[h1][64:128, lo:hi], ps[64:128, 0:hi - lo])
            if half == 0 and p > 0:
                nc.sync.dma_start(kpt[h0][64:128, :], ebt[h0 * DK:(h0 + 1) * DK, :])
                nc.sync.dma_start(kpt[h1][0:64, :], ebt[h1 * DK:(h1 + 1) * DK, :])

        def v_unit(ni, half):
            def f():
                lo, hi = (0, 512) if half == 0 else (512, TOTAL)
                ps = proj_ps.tile([128, 512], F32, tag="proj", name="proj_v")
                for ec in range(EC):
                    nc.tensor.matmul(ps[:, 0:hi - lo],
                                     lhsT=xte_sb[ec][:, ni * 128:(ni + 1) * 128],
                                     rhs=wv_sb[ec][:, lo:hi], start=(ec == 0), stop=(ec == EC - 1))
                hlo, hhi = lo // DK, hi // DK
                nc.vector.tensor_copy(
                    vsb[ni][:, hlo:hhi, 0:DK],
                    ps[:, 0:hi - lo].rearrange("p (h d) -> p h d", d=DK))
                if half == 1:
                    nc.vector.memset(vsb[ni][:, :, DK], 1.0)
            return (1.3, f)

        def av_unit(h, attns, g):
            def f():
                emit_av(h, attns, g)
            return (0.5, f)

        def pump(budget):
            while units and budget > 0:
                c, f = units.popleft()
                f()
                budget -= c

        # kt pair0 + qt pair0 emitted up front (head 0 critical path)
        emit_kt_half(0, 0)
        emit_kt_half(0, 1)
        pair_ready[0] = 2
        for mh in range(4):
            qt_unit(0, mh)[1]()
        for ni in range(NI):
            units.append(v_unit(ni, 0))
            units.append(v_unit(ni, 1))

        slot = 0
        for h in range(H):
            p = h // 2
            if h % 2 == 1 and p + 1 <= PAIRS - 1:
                # next pair's projections jump the queue so the even-head
                # boundary never has to force-drain a big batch
                for mh in range(3, -1, -1):
                    units.appendleft(qt_unit(p + 1, mh))
                units.appendleft(kt_unit(p + 1, 1))
                units.appendleft(kt_unit(p + 1, 0))
            while pair_ready.get(p, 0) < 6:
                c, f = units.popleft()
                f()
            attns = []
            for ni in range(NI):
                at = attn_pool.tile([128, M], BF16, tag="attn", name="attn_t")
                attns.append(at)
                for half in range(2):
                    emit_scores_exp_half(h, qts[h], ni, half, at)
                    pump(2.5 if slot < 12 else 0.75)
                slot += 1
            qts[h] = None  # allow qpt slot reuse
            for g in range(8):
                units.append(av_unit(h, attns, g))
        while units:
            c, f = units.popleft()
            f()

    nc.compile()
    return nc


def _get_nc():
    if "nc" not in _CACHE:
        _CACHE["nc"] = _build()
    return _CACHE["nc"]


def kernel(**inputs):
    global LAST_EXEC_NS, LAST_TRACE_DIR
    from concourse.bass_utils import run_bass_kernel_spmd

    ehr = np.asarray(inputs["ehr_embeddings"], dtype=np.float32)
    mi = np.asarray(inputs["missing_indices"]).astype(np.int64)
    ei = np.asarray(inputs["exist_indices"]).astype(np.int64)
    Wq = np.asarray(inputs["Wq"], dtype=np.float32)
    Wk = np.asarray(inputs["Wk"], dtype=np.float32)
    Wv = np.asarray(inputs["Wv"], dtype=np.float32)
    bq = np.asarray(inputs["bq"], dtype=np.float32)
    bv = np.asarray(inputs["bv"], dtype=np.float32)
    cooc = np.asarray(inputs["cooc_bias"], dtype=np.float32)
    # bk is softmax-shift-invariant (adds a per-query constant to scores);
    # dropped on device, consistent across cores so the combine is exact.

    scale = 1.0 / np.sqrt(np.float32(DK))
    wq_s = np.ascontiguousarray((Wq * scale).astype(np.float16))
    bq_s = np.ascontiguousarray((bq * scale).reshape(PAIRS, 128).T)

    missing_emb = ehr[mi]                       # [M, E]
    xt_m = np.ascontiguousarray(missing_emb.T.astype(np.float16))  # [E, M]
    mbt = np.ascontiguousarray(
        cooc[:, mi, :].transpose(0, 2, 1).reshape(H * DK, M).astype(np.float16))

    common = {"xt_m": xt_m, "mbt": mbt, "wq": wq_s,
              "wk": np.ascontiguousarray(Wk.astype(np.float16)),
              "wv": np.ascontiguousarray(Wv.astype(np.float16)), "bq": bq_s}
    in_maps = []
    for c in range(CORES):
        eic = ei[c * NLOC:(c + 1) * NLOC]
        xt_e = np.ascontiguousarray(ehr[eic].T.astype(np.float16))  # [E, NLOC]
        ebt = np.ascontiguousarray(
            cooc[:, eic, :].transpose(0, 2, 1).reshape(H * DK, NLOC).astype(np.float16))
        in_maps.append({**common, "xt_e": xt_e, "ebt": ebt})

    nc = _get_nc()
    trace = os.environ.get("KERNEL_TRACE") == "1"
    kwargs = {}
    if trace:
        import tempfile
        LAST_TRACE_DIR = tempfile.mkdtemp(prefix="kern_trace_")
        kwargs = {"trace": True, "tmpdir": LAST_TRACE_DIR}
        try:
            import ntff_shim
            ntff_shim.install()
        except ImportError:
            pass
    res = run_bass_kernel_spmd(nc, in_maps, list(range(CORES)), **kwargs)
    LAST_EXEC_NS = res.exec_time_ns

    # ---- host combine ----
    num = np.zeros((H, M, DK), dtype=np.float64)
    den = np.zeros((H, M), dtype=np.float64)
    for c in range(CORES):
        op = res.results[c]["out_p"].astype(np.float64)  # [H, M, DK+1]
        num += op[:, :, :DK]
        den += op[:, :, DK]
    out = num / den[:, :, None]                          # [H, M, DK]
    out = out.transpose(1, 0, 2).reshape(M, TOTAL) + bv.astype(np.float64)
    result = ehr.copy()
    result[mi] = out.astype(np.float32)
    return result
